# revision 7
# baseline (speedup 1.0000x reference)
"""Trainium2 Bass kernel for a dense transformer block (self-contained).

Block: x + attn(x) -> rmsnorm -> + swiglu-mlp -> rmsnorm
Shapes: B=2, S=2048, D=2048, H=16 (hd=128), HIDDEN=5632, fp32.

Sharding over 8 NeuronCores:
  - Attention head-parallel: core i computes heads 2i, 2i+1 for both batches
    from a replicated transposed-x. One AllToAll (4.2MB/rank) redistributes
    the attention context from head-sharded to token-sharded.
  - wo projection, rmsnorms and the MLP are token-parallel (512 tokens/core,
    full weights). All post-attention activations are kept feature-major
    [feature_partition, token_free]; rmsnorm partition reductions are done
    with ones-matmuls on the PE, broadcasts with K=1 matmuls.
  - All matmuls run as float32r (tf32-like single-pass).
"""
import os
import numpy as np

import concourse.bacc as bacc
import concourse.bass as bass
import concourse.tile as tile
import concourse.mybir as mybir

F32 = mybir.dt.float32
F32R = mybir.dt.float32r
AF = mybir.ActivationFunctionType

NCORES = 8
B, S, D = 2, 2048, 2048
H, HD = 16, 128
HID = 5632
NT = B * S              # 4096 tokens global
TPC = NT // NCORES      # 512 tokens per core
HPC = H // NCORES       # 2 heads per core
KD = D // 128           # 16 feature chunks
KH = HID // 128         # 44 hidden chunks
NJ = NT // 512          # 8 global token chunks of 512
QC = S // 512           # 4 q-chunks per batch
EPS = 1e-6
ISQ = 1.0 / np.sqrt(HD)

DEBUG = bool(int(os.environ.get("KERNEL_DEBUG", "0")))

_CACHE = {}


# --------------------------------------------------------------------------
# device program
# --------------------------------------------------------------------------

def _build_nc():
    nc = bacc.Bacc("TRN2", target_bir_lowering=False, debug=False,
                   num_devices=NCORES)

    # inputs (per-core views prepared on host)
    xT = nc.dram_tensor("xT", [D, NT], F32R, kind="ExternalInput")
    xtsl = nc.dram_tensor("xtsl", [D, TPC], F32R, kind="ExternalInput")
    wq = nc.dram_tensor("wq", [D, HPC * HD], F32R, kind="ExternalInput")
    wk = nc.dram_tensor("wk", [D, HPC * HD], F32R, kind="ExternalInput")
    wv = nc.dram_tensor("wv", [D, HPC * HD], F32R, kind="ExternalInput")
    wo = nc.dram_tensor("wo", [D, D], F32R, kind="ExternalInput")
    w1T = nc.dram_tensor("w1T", [D, HID], F32R, kind="ExternalInput")
    v1T = nc.dram_tensor("v1T", [D, HID], F32R, kind="ExternalInput")
    w2T = nc.dram_tensor("w2T", [HID, D], F32R, kind="ExternalInput")
    atab = nc.dram_tensor("atab", [HD, S], F32R, kind="ExternalInput")
    btab = nc.dram_tensor("btab", [HD, S], F32R, kind="ExternalInput")
    rmat = nc.dram_tensor("rmat", [HD, HD], F32R, kind="ExternalInput")
    masks = nc.dram_tensor("masks", [4, 128, 512], F32R, kind="ExternalInput")
    onesk = nc.dram_tensor("onesk", [128, 1], F32R, kind="ExternalInput")
    onesm = nc.dram_tensor("onesm", [1, 128], F32R, kind="ExternalInput")
    n1w = nc.dram_tensor("n1w", [128, KD], F32, kind="ExternalInput")
    n2w = nc.dram_tensor("n2w", [128, KD], F32, kind="ExternalInput")

    out = nc.dram_tensor("out", [D, TPC], F32, kind="ExternalOutput")
    if DEBUG:
        dbg_o = nc.dram_tensor("dbg_o", [D, TPC], F32, kind="ExternalOutput")
        dbg_y = nc.dram_tensor("dbg_y", [D, TPC], F32, kind="ExternalOutput")

    a2a_in = nc.dram_tensor("a2a_in", [NCORES, HPC * HD, TPC], F32R)
    a2a_out = nc.dram_tensor("a2a_out", [NCORES, HPC * HD, TPC], F32R)

    with tile.TileContext(nc) as tc:
        # ---- persistent constants ------------------------------------
        const = tc.alloc_tile_pool(name="const", bufs=1)
        onesk_sb = const.tile([128, 1], F32R, tag="onesk")
        nc.sync.dma_start(onesk_sb[:], onesk[:])
        onesm_sb = const.tile([1, 128], F32R, tag="onesm")
        nc.sync.dma_start(onesm_sb[:], onesm[:])
        n1w_sb = const.tile([128, KD], F32, tag="n1w")
        nc.sync.dma_start(n1w_sb[:], n1w[:])
        n2w_sb = const.tile([128, KD], F32, tag="n2w")
        nc.sync.dma_start(n2w_sb[:], n2w[:])
        epsc = const.tile([1, 1], F32, tag="epsc")
        nc.vector.memset(epsc[:], EPS)

        # ---- attention-persistent data -------------------------------
        attn_pool = tc.alloc_tile_pool(name="attn", bufs=1)
        # q^T, k^T: [hd=128, 512] per (head, global-chunk j); roped, permuted d
        qT = [[attn_pool.tile([128, 512], F32R, tag=f"qT{h}_{j}", name=f"qT{h}_{j}")
               for j in range(NJ)] for h in range(HPC)]
        kT = [[attn_pool.tile([128, 512], F32R, tag=f"kT{h}_{j}", name=f"kT{h}_{j}")
               for j in range(NJ)] for h in range(HPC)]
        # v natural: [tok 128, 2 heads * 128] per global token tile g
        vN = [attn_pool.tile([128, HPC * HD], F32R, tag=f"v{g}", name=f"v{g}")
              for g in range(NT // 128)]

        rope_pool = tc.alloc_tile_pool(name="rope", bufs=1)
        rmat_sb = rope_pool.tile([HD, HD], F32R, tag="rmat")
        nc.sync.dma_start(rmat_sb[:], rmat[:])
        atab_sb = rope_pool.tile([HD, S], F32R, tag="atab")
        nc.sync.dma_start(atab_sb[:], atab[:])
        btab_sb = rope_pool.tile([HD, S], F32R, tag="btab")
        nc.sync.dma_start(btab_sb[:], btab[:])
        # qkv weight slices, laid out [128, (k, n)]
        wq_sb = rope_pool.tile([128, KD * HPC * HD], F32R, tag="wq")
        nc.sync.dma_start(
            wq_sb[:].rearrange("p (k n) -> p k n", k=KD),
            wq[:].rearrange("(k p) n -> p k n", p=128))
        wk_sb = rope_pool.tile([128, KD * HPC * HD], F32R, tag="wk")
        nc.sync.dma_start(
            wk_sb[:].rearrange("p (k n) -> p k n", k=KD),
            wk[:].rearrange("(k p) n -> p k n", p=128))
        wv_sb = rope_pool.tile([128, KD * HPC * HD], F32R, tag="wv")
        nc.sync.dma_start(
            wv_sb[:].rearrange("p (k n) -> p k n", k=KD),
            wv[:].rearrange("(k p) n -> p k n", p=128))

        # ================= Phase A1: q^T, k^T + RoPE ==================
        # Sweep 1: for each 512-token chunk, accumulate q/k for both heads
        # (4 PSUM banks), then rope. xT tiles rotate through 4 buffers.
        with (
            tc.tile_pool(name="xTs", bufs=4) as xts_pool,
            tc.tile_pool(name="ascr", bufs=3) as ascr,
            tc.tile_pool(name="psqk", bufs=1, space="PSUM") as psqk,
            tc.tile_pool(name="psr", bufs=2, space="PSUM") as psr_pool,
        ):
            for j in range(NJ):
                sloc = (j % QC) * 512  # position within batch
                acc = {}
                for h in range(HPC):
                    for w in ("q", "k"):
                        acc[(w, h)] = psqk.tile([128, 512], F32, tag=f"ps{w}{h}", name=f"ps{w}{h}")
                for k in range(KD):
                    xt = xts_pool.tile([128, 512], F32R, tag="x")
                    nc.sync.dma_start(
                        xt[:], xT[128 * k:128 * (k + 1),
                                  512 * j:512 * (j + 1)])
                    for h in range(HPC):
                        for w, wsb in (("q", wq_sb), ("k", wk_sb)):
                            lhs = wsb[:, k * HPC * HD + 128 * h:
                                      k * HPC * HD + 128 * (h + 1)]
                            nc.tensor.matmul(acc[(w, h)][:], lhs, xt[:],
                                             start=(k == 0),
                                             stop=(k == KD - 1))
                for h in range(HPC):
                    for w, dest in (("q", qT), ("k", kT)):
                        ps = acc[(w, h)]
                        # rope: out = raw*A + (R @ raw)*B
                        raw = ascr.tile([128, 512], F32R, tag="raw")
                        nc.scalar.activation(raw[:], ps[:], AF.Copy)
                        psr = psr_pool.tile([128, 512], F32, tag="psr")
                        nc.tensor.matmul(psr[:], rmat_sb[:], raw[:],
                                         start=True, stop=True)
                        t1 = ascr.tile([128, 512], F32R, tag="t1")
                        nc.vector.tensor_mul(
                            t1[:], raw[:], atab_sb[:, sloc:sloc + 512])
                        t2 = ascr.tile([128, 512], F32R, tag="t2")
                        nc.vector.tensor_mul(
                            t2[:], btab_sb[:, sloc:sloc + 512], psr[:])
                        dtile = dest[h][j]
                        nc.vector.tensor_add(dtile[:], t1[:], t2[:])

        # ================= Phase A2: v natural ========================
        with (
            tc.tile_pool(name="xTs2", bufs=4) as xts2_pool,
            tc.tile_pool(name="psv", bufs=1, space="PSUM") as psv_pool,
        ):
            for j in range(NJ):
                psv = [psv_pool.tile([128, HPC * HD], F32, tag=f"psv{t}", name=f"psv{t}")
                       for t in range(4)]
                for k in range(KD):
                    xt = xts2_pool.tile([128, 512], F32R, tag="x2")
                    nc.sync.dma_start(
                        xt[:], xT[128 * k:128 * (k + 1),
                                  512 * j:512 * (j + 1)])
                    for t in range(4):
                        lhs = xt[:, 128 * t:128 * (t + 1)]
                        nc.tensor.matmul(
                            psv[t][:], lhs,
                            wv_sb[:, k * HPC * HD:(k + 1) * HPC * HD],
                            start=(k == 0), stop=(k == KD - 1))
                for t in range(4):
                    nc.scalar.activation(vN[4 * j + t][:], psv[t][:], AF.Copy)

        rope_pool.release()

        # ================= Phase B: attention =========================
        with (
            tc.tile_pool(name="mask", bufs=1) as mask_pool,
            tc.tile_pool(name="probs", bufs=6) as probs_pool,
            tc.tile_pool(name="bscr", bufs=3) as bscr,
            tc.tile_pool(name="pss", bufs=3, space="PSUM") as pss_pool,
            tc.tile_pool(name="psd", bufs=2, space="PSUM") as psd_pool,
            tc.tile_pool(name="pso", bufs=2, space="PSUM") as pso_pool,
            tc.tile_pool(name="psb", bufs=1, space="PSUM") as psb_pool,
        ):
            mask_sb = [mask_pool.tile([128, 512], F32R, tag=f"m{m}", name=f"m{m}")
                       for m in range(4)]
            for m in range(4):
                nc.sync.dma_start(mask_sb[m][:], masks[m])

            for b in range(B):
                for h in range(HPC):
                    for qc in range(QC):
                        j = QC * b + qc
                        nkt = 4 * (qc + 1)
                        psd = psd_pool.tile([1, 512], F32, tag="psd")
                        pso = pso_pool.tile([128, 512], F32, tag="pso")
                        for kt in range(nkt):
                            jk = QC * b + kt // 4
                            ksl = kT[h][jk][:, 128 * (kt % 4):
                                            128 * (kt % 4 + 1)]
                            pss = pss_pool.tile([128, 512], F32, tag="pss")
                            nc.tensor.matmul(pss[:], ksl, qT[h][j][:],
                                             start=True, stop=True)
                            prob = probs_pool.tile([128, 512], F32R, tag="pr")
                            m = kt - 4 * qc
                            if m >= 0:
                                escr = bscr.tile([128, 512], F32R, tag="e")
                                nc.scalar.activation(escr[:], pss[:], AF.Exp,
                                                     scale=ISQ)
                                nc.vector.tensor_mul(prob[:], escr[:],
                                                     mask_sb[m][:])
                            else:
                                nc.scalar.activation(prob[:], pss[:], AF.Exp,
                                                     scale=ISQ)
                            nc.tensor.matmul(psd[:], onesk_sb[:], prob[:],
                                             start=(kt == 0),
                                             stop=(kt == nkt - 1))
                            g = 16 * b + kt
                            vsl = vN[g][:, 128 * h:128 * (h + 1)]
                            nc.tensor.matmul(pso[:], vsl, prob[:],
                                             start=(kt == 0),
                                             stop=(kt == nkt - 1))
                        # normalize: o / colsum, then ship to a2a buffer
                        rd = bscr.tile([1, 512], F32R, tag="rd")
                        with nc.allow_low_precision(reason="f32r softmax recip"):
                            nc.vector.reciprocal(rd[:], psd[:])
                        psb = psb_pool.tile([128, 512], F32, tag="psb")
                        nc.tensor.matmul(psb[:], onesm_sb[:], rd[:],
                                         start=True, stop=True)
                        rb = bscr.tile([128, 512], F32R, tag="rb")
                        nc.scalar.activation(rb[:], psb[:], AF.Copy)
                        osb = bscr.tile([128, 512], F32R, tag="osb")
                        nc.vector.tensor_mul(osb[:], rb[:], pso[:])
                        nc.sync.dma_start(
                            a2a_in[j, 128 * h:128 * (h + 1), :], osb[:])

        attn_pool.release()

        # ================= Phase C: AllToAll ==========================
        nc.gpsimd.collective_compute(
            "AllToAll", mybir.AluOpType.bypass,
            replica_groups=[list(range(NCORES))],
            ins=[a2a_in[:].opt()], outs=[a2a_out[:].opt()],
        )

        # ================= Phase D: wo + residual + rmsnorm ===========
        post_pool = tc.alloc_tile_pool(name="post", bufs=1)
        yt = [post_pool.tile([128, 512], F32R, tag=f"y{m}", name=f"ymt{m}") for m in range(KD)]

        with (
            tc.tile_pool(name="oT", bufs=1) as oT_pool,
            tc.tile_pool(name="wos", bufs=2) as wo_pool,
            tc.tile_pool(name="ht", bufs=1) as ht_pool,
            tc.tile_pool(name="dscr", bufs=3) as dscr,
            tc.tile_pool(name="psh", bufs=2, space="PSUM") as psh_pool,
            tc.tile_pool(name="psn", bufs=2, space="PSUM") as psn_pool,
        ):
            oT = []
            for r in range(KD):
                ot = oT_pool.tile([128, 512], F32R, tag=f"o{r}", name=f"oTt{r}")
                lo = 128 * (r % 2)
                nc.sync.dma_start(ot[:], a2a_out[r // 2, lo:lo + 128, :])
                oT.append(ot)
            xsl = []
            for m in range(KD):
                xs = ht_pool.tile([128, 512], F32R, tag=f"xs{m}", name=f"xs{m}")
                nc.sync.dma_start(xs[:], xtsl[128 * m:128 * (m + 1), :])
                xsl.append(xs)

            ht = []
            psss = psn_pool.tile([1, 512], F32, tag="ss")
            for m in range(KD):
                wos = wo_pool.tile([128, KD * 128], F32R, tag="wos")
                nc.sync.dma_start(
                    wos[:].rearrange("p (r n) -> p r n", r=KD),
                    wo[:, 128 * m:128 * (m + 1)]
                    .rearrange("(r p) n -> p r n", p=128))
                psh = psh_pool.tile([128, 512], F32, tag="psh")
                for r in range(KD):
                    nc.tensor.matmul(psh[:], wos[:, 128 * r:128 * (r + 1)],
                                     oT[r][:],
                                     start=(r == 0), stop=(r == KD - 1))
                h_sb = ht_pool.tile([128, 512], F32R, tag=f"h{m}", name=f"hmt{m}")
                nc.vector.tensor_add(h_sb[:], xsl[m][:], psh[:])
                ht.append(h_sb)
                sq = dscr.tile([128, 512], F32R, tag="sq")
                nc.vector.tensor_mul(sq[:], h_sb[:], h_sb[:])
                nc.tensor.matmul(psss[:], onesk_sb[:], sq[:],
                                 start=(m == 0), stop=(m == KD - 1))

            # scale = 1/sqrt(mean+eps), broadcast to 128 partitions
            u = dscr.tile([1, 512], F32, tag="u")
            nc.scalar.activation(u[:], psss[:], AF.Sqrt, scale=1.0 / D,
                                 bias=epsc[:])
            rs = dscr.tile([1, 512], F32R, tag="rs")
            with nc.allow_low_precision(reason="f32r rmsnorm recip"):
                nc.vector.reciprocal(rs[:], u[:])
            psb1 = psn_pool.tile([128, 512], F32, tag="bc")
            nc.tensor.matmul(psb1[:], onesm_sb[:], rs[:], start=True, stop=True)
            rb1 = dscr.tile([128, 512], F32R, tag="rb1")
            nc.scalar.activation(rb1[:], psb1[:], AF.Copy)
            for m in range(KD):
                ytmp = dscr.tile([128, 512], F32R, tag="ytmp")
                nc.vector.tensor_mul(ytmp[:], ht[m][:], rb1[:])
                nc.scalar.activation(yt[m][:], ytmp[:], AF.Copy,
                                     scale=n1w_sb[:, m:m + 1])
                if DEBUG:
                    nc.sync.dma_start(
                        dbg_o[128 * m:128 * (m + 1), :],
                        ht[m][:].bitcast(F32))
                    nc.sync.dma_start(
                        dbg_y[128 * m:128 * (m + 1), :],
                        yt[m][:].bitcast(F32))

        # ================= Phase E: MLP ===============================
        mlp_pool = tc.alloc_tile_pool(name="mlp", bufs=1)
        mt = [mlp_pool.tile([128, 512], F32R, tag=f"mm{t}", name=f"mmt{t}") for t in range(KH)]
        with (
            tc.tile_pool(name="w1s", bufs=2) as w1_pool,
            tc.tile_pool(name="v1s", bufs=2) as v1_pool,
            tc.tile_pool(name="escr", bufs=3) as escr_pool,
            tc.tile_pool(name="ps1", bufs=2, space="PSUM") as ps1_pool,
            tc.tile_pool(name="ps2", bufs=2, space="PSUM") as ps2_pool,
        ):
            for t in range(KH):
                w1s = w1_pool.tile([128, KD * 128], F32R, tag="w1s")
                nc.sync.dma_start(
                    w1s[:].rearrange("p (k n) -> p k n", k=KD),
                    w1T[:, 128 * t:128 * (t + 1)]
                    .rearrange("(k p) n -> p k n", p=128))
                v1s = v1_pool.tile([128, KD * 128], F32R, tag="v1s")
                nc.sync.dma_start(
                    v1s[:].rearrange("p (k n) -> p k n", k=KD),
                    v1T[:, 128 * t:128 * (t + 1)]
                    .rearrange("(k p) n -> p k n", p=128))
                ps1 = ps1_pool.tile([128, 512], F32, tag="ps1")
                ps2 = ps2_pool.tile([128, 512], F32, tag="ps2")
                for k in range(KD):
                    nc.tensor.matmul(ps1[:], w1s[:, 128 * k:128 * (k + 1)],
                                     yt[k][:],
                                     start=(k == 0), stop=(k == KD - 1))
                for k in range(KD):
                    nc.tensor.matmul(ps2[:], v1s[:, 128 * k:128 * (k + 1)],
                                     yt[k][:],
                                     start=(k == 0), stop=(k == KD - 1))
                ssc = escr_pool.tile([128, 512], F32R, tag="ssc")
                nc.scalar.activation(ssc[:], ps1[:], AF.Silu)
                nc.vector.tensor_mul(mt[t][:], ssc[:], ps2[:])

        # ================= Phase E2: down-proj + rmsnorm ==============
        with (
            tc.tile_pool(name="w2s", bufs=2) as w2_pool,
            tc.tile_pool(name="fscr", bufs=3) as fscr,
            tc.tile_pool(name="pso2", bufs=2, space="PSUM") as pso2_pool,
            tc.tile_pool(name="psn2", bufs=2, space="PSUM") as psn2_pool,
        ):
            psss2 = psn2_pool.tile([1, 512], F32, tag="ss2")
            for m in range(KD):
                w2s = w2_pool.tile([128, KH * 128], F32R, tag="w2s")
                nc.sync.dma_start(
                    w2s[:].rearrange("p (t n) -> p t n", t=KH),
                    w2T[:, 128 * m:128 * (m + 1)]
                    .rearrange("(t p) n -> p t n", p=128))
                pso2 = pso2_pool.tile([128, 512], F32, tag="pso2")
                for t in range(KH):
                    nc.tensor.matmul(pso2[:], w2s[:, 128 * t:128 * (t + 1)],
                                     mt[t][:],
                                     start=(t == 0), stop=(t == KH - 1))
                # residual2 overwrites y tile (no later reader)
                nc.vector.tensor_add(yt[m][:], yt[m][:], pso2[:])
                sq2 = fscr.tile([128, 512], F32R, tag="sq2")
                nc.vector.tensor_mul(sq2[:], yt[m][:], yt[m][:])
                nc.tensor.matmul(psss2[:], onesk_sb[:], sq2[:],
                                 start=(m == 0), stop=(m == KD - 1))

            u2 = fscr.tile([1, 512], F32, tag="u2")
            nc.scalar.activation(u2[:], psss2[:], AF.Sqrt, scale=1.0 / D,
                                 bias=epsc[:])
            rs2 = fscr.tile([1, 512], F32R, tag="rs2")
            with nc.allow_low_precision(reason="f32r rmsnorm recip"):
                nc.vector.reciprocal(rs2[:], u2[:])
            psb2 = psn2_pool.tile([128, 512], F32, tag="bc2")
            nc.tensor.matmul(psb2[:], onesm_sb[:], rs2[:], start=True,
                             stop=True)
            rb2 = fscr.tile([128, 512], F32R, tag="rb2")
            nc.scalar.activation(rb2[:], psb2[:], AF.Copy)
            for m in range(KD):
                ftmp = fscr.tile([128, 512], F32R, tag="ftmp")
                nc.vector.tensor_mul(ftmp[:], yt[m][:], rb2[:])
                fout = fscr.tile([128, 512], F32, tag="fout")
                nc.scalar.activation(fout[:], ftmp[:], AF.Copy,
                                     scale=n2w_sb[:, m:m + 1])
                nc.sync.dma_start(out[128 * m:128 * (m + 1), :], fout[:])

        mlp_pool.release()
        post_pool.release()
        const.release()

    nc.compile()
    return nc


# --------------------------------------------------------------------------
# host-side: shard inputs, run, gather
# --------------------------------------------------------------------------

def _prep_inputs(x, wq, wk, wv, wo, w_mlp, v_mlp, w2_mlp,
                 norm1_w, norm2_w, freqs_cos, freqs_sin):
    f32 = np.float32
    xf = np.asarray(x, f32).reshape(NT, D)
    xT = np.ascontiguousarray(xf.T)

    perm = np.concatenate([np.arange(0, HD, 2), np.arange(1, HD, 2)])
    cosT = np.asarray(freqs_cos, f32).T          # [64, S]
    sinT = np.asarray(freqs_sin, f32).T
    atab = np.ascontiguousarray(np.concatenate([cosT, cosT], axis=0))
    btab = np.ascontiguousarray(np.concatenate([-sinT, sinT], axis=0))
    rmat = np.zeros((HD, HD), f32)
    e = np.arange(64)
    rmat[e, 64 + e] = 1.0
    rmat[64 + e, e] = 1.0

    m_idx = np.arange(4)[:, None, None]
    p_idx = np.arange(128)[None, :, None]
    f_idx = np.arange(512)[None, None, :]
    masks = (128 * m_idx + p_idx <= f_idx).astype(f32)

    n1w = np.ascontiguousarray(
        np.asarray(norm1_w, f32).reshape(KD, 128).T)
    n2w = np.ascontiguousarray(
        np.asarray(norm2_w, f32).reshape(KD, 128).T)

    wq = np.asarray(wq, f32)
    wk = np.asarray(wk, f32)
    wv = np.asarray(wv, f32)
    wo_c = np.ascontiguousarray(np.asarray(wo, f32))
    w1T = np.ascontiguousarray(np.asarray(w_mlp, f32).T)
    v1T = np.ascontiguousarray(np.asarray(v_mlp, f32).T)
    w2T = np.ascontiguousarray(np.asarray(w2_mlp, f32).T)
    onesk = np.ones((128, 1), f32)
    onesm = np.ones((1, 128), f32)

    in_maps = []
    for i in range(NCORES):
        cols_p = []   # permuted columns for q,k
        cols_n = []   # natural columns for v
        for p in range(HPC):
            h = HPC * i + p
            cols_p.extend(h * HD + perm)
            cols_n.extend(range(h * HD, (h + 1) * HD))
        in_maps.append({
            "xT": xT,
            "xtsl": np.ascontiguousarray(xT[:, TPC * i:TPC * (i + 1)]),
            "wq": np.ascontiguousarray(wq[:, cols_p]),
            "wk": np.ascontiguousarray(wk[:, cols_p]),
            "wv": np.ascontiguousarray(wv[:, cols_n]),
            "wo": wo_c, "w1T": w1T, "v1T": v1T, "w2T": w2T,
            "atab": atab, "btab": btab, "rmat": rmat, "masks": masks,
            "onesk": onesk, "onesm": onesm, "n1w": n1w, "n2w": n2w,
        })
    return in_maps


def _get_runner():
    """Build (once) the compiled SPMD executable; returns a callable
    taking in_maps and returning per-core output dicts."""
    if "runner" in _CACHE:
        return _CACHE["runner"]

    nc = _build_nc()

    import jax
    from jax.sharding import Mesh, PartitionSpec
    from jax.experimental.shard_map import shard_map
    from concourse import bass2jax
    from concourse.bass2jax import (_bass_exec_p, install_neuronx_cc_hook,
                                    partition_id_tensor)

    install_neuronx_cc_hook()

    partition_name = (nc.partition_id_tensor.name
                      if nc.partition_id_tensor else None)
    in_names, out_names, out_avals = [], [], []
    for alloc in nc.m.functions[0].allocations:
        if not isinstance(alloc, mybir.MemoryLocationSet):
            continue
        name = alloc.memorylocations[0].name
        if alloc.kind == "ExternalInput":
            if name != partition_name:
                in_names.append(name)
        elif alloc.kind == "ExternalOutput":
            out_names.append(name)
            out_avals.append(jax.core.ShapedArray(
                tuple(alloc.tensor_shape), mybir.dt.np(alloc.dtype)))
    n_params = len(in_names)
    all_in_names = list(in_names + out_names)
    if partition_name is not None:
        all_in_names.append(partition_name)
    all_in_names = tuple(all_in_names)

    def _body(*args):
        operands = list(args)
        if partition_name is not None:
            operands.append(partition_id_tensor())
        outs = _bass_exec_p.bind(
            *operands,
            out_avals=tuple(out_avals),
            in_names=all_in_names,
            out_names=tuple(out_names),
            lowering_input_output_aliases=(),
            sim_require_finite=True,
            sim_require_nnan=True,
            nc=nc,
        )
        return tuple(outs)

    devices = jax.devices()[:NCORES]
    mesh = Mesh(np.asarray(devices), ("core",))
    nio = n_params + len(out_names)
    sharded = jax.jit(
        shard_map(_body, mesh=mesh,
                  in_specs=(PartitionSpec("core"),) * nio,
                  out_specs=(PartitionSpec("core"),) * len(out_names),
                  check_rep=False),
        keep_unused=True,
    )

    zero_outs = [np.zeros((NCORES * a.shape[0],) + a.shape[1:], a.dtype)
                 for a in out_avals]

    def make_args(in_maps):
        concat_in = [
            np.concatenate([np.asarray(in_maps[c][n]) for c in range(NCORES)],
                           axis=0)
            for n in in_names
        ]
        return concat_in + zero_outs

    def run(args):
        out_arrs = sharded(*args)
        return [
            {name: np.asarray(out_arrs[k]).reshape(
                NCORES, *out_avals[k].shape)[c]
             for k, name in enumerate(out_names)}
            for c in range(NCORES)
        ]

    _CACHE["runner"] = (run, make_args, sharded)
    return _CACHE["runner"]


def kernel(**inputs) -> np.ndarray:
    run, make_args, _ = _get_runner()
    in_maps = _prep_inputs(**inputs)
    results = run(make_args(in_maps))
    fullT = np.concatenate([results[i]["out"] for i in range(NCORES)], axis=1)
    out = fullT.T.reshape(B, S, D)
    if DEBUG:
        kernel._last_results = results
    return np.ascontiguousarray(out)
